# revision 11
# baseline (speedup 1.0000x reference)
"""Bass/Trainium2 kernel for nn_BiChannelAttention (single-query local-window attention).

Math (per batch b, head h, with S=2049, window W=256, cutoff=S-W=1793):
  Positions before the cutoff receive a -1e6 additive mask, so after softmax their
  weight is exactly 0.0 in fp32 (exp underflows). Only the last W positions matter.
  The reference's masked_fill sequence (1->0, then every 0->NEG) sets ALL positions
  to NEG -- a uniform shift softmax cancels, so time_mask is a no-op.
  bk shifts every score of a batch equally (q . bk) -- cancelled by softmax.
  bv contributes exactly bv to the output (attn weights sum to 1) -> folded into
  the residual constant on the host.

Per (b, h): window rows X [W=256, 128] (last 255 cache rows + content row):
    q   = cnt_h @ Wq_h + bq                  (128)
    kq  = (Wk_h/sqrt(128))^T q               (128)   <- folds Wk into q
    sc  = X kq - pos_param * bucket(s)       (256)
    a   = exp(sc)      (no max-subtraction: unmasked scores are O(1))
    xa  = X^T a ;  den = sum(a)
    out = (Wv_h^T xa) / den + cnt_h + bv_h   (128)

Sharding: tensor-parallel over heads, 2 heads per core x 8 cores.

Engine/latency plan per core:
  - 6 input DMAs fired in parallel at t=0 across 4 queues (sync/scalar/vector/gpsimd)
    so the serialized ~1-2us HBM fixed costs overlap.
  - X uploaded twice in fp8e4 (score layout X^T [j, d, (b,s)] and value layout
    [s128, (j,b,st,d)]), ~1MB/core; weights bf16, residual/bias consts fp32.
  - PE warm-up: dummy matmuls during the DMA window lift the HAM clock gate
    (1.2 -> 2.4 GHz) before the real matmuls arrive.
  - q/kq for both heads in one K-stacked accumulation chain (zero-padded rhs).
  - scores: 32 matmuls, stationary = 128-col fp8 X^T tile, moving = kq [128,8]
    -> psum [s128, 8 batches x 8 cols]; the useful column of block b is col 9b.
  - exp+bias fused in 4 scalar.activation ops reading the strided psum columns.
  - denominator: ones-matmul broadcast; normalization deferred to the output.
  - value: 32 matmuls, stationary = fp8 X tile, moving = one attn column.
"""

import sys
import numpy as np
import ml_dtypes

for _p in ("/opt/trn_rl_repo", "/root/.axon_site/_ro/trn_rl_repo"):
    if _p not in sys.path:
        sys.path.insert(0, _p)

import concourse.bass as bass
import concourse.bacc as bacc
import concourse.mybir as mybir
from concourse.tile import TileContext
from concourse.bass_utils import run_bass_kernel_spmd
from concourse import bass_utils as _bass_utils

# The walrus NEFF epilogue clears every allocatable semaphore one instruction
# at a time (~6us of the measured exec window). Capping the allocator shrinks
# that epilogue; this kernel's Tile schedule needs only a few dozen sems.
if not getattr(_bass_utils, "_max_sem_patched", False):
    _orig_walrus_args = _bass_utils.get_walrus_args

    def _walrus_args_capped(*a, **k):
        return _orig_walrus_args(*a, **k) + ["--max-sem-num=64"]

    _bass_utils.get_walrus_args = _walrus_args_capped
    _bass_utils._max_sem_patched = True

F32 = mybir.dt.float32
BF16 = mybir.dt.bfloat16
FP8 = mybir.dt.float8e4
NP_FP8 = ml_dtypes.float8_e4m3
NP_BF16 = ml_dtypes.bfloat16

P = 128          # partitions / head_dim
B = 8            # batch
H = 16           # heads total
HPC = 2          # heads per core
NCORES = 8
T = 2048
S = T + 1
W = 256          # local attention window
CUTOFF = S - W   # 1793
EXP = mybir.ActivationFunctionType.Exp
N_WARM = 22      # dummy matmuls to lift the HAM clock gate during the DMA window

_NC_CACHE = {}


def _build_nc():
    nc = bacc.Bacc(None, target_bir_lowering=False, debug=False)
    xt_in = nc.declare_dram_parameter("xt", [HPC, P, B * W], FP8, isOutput=False)
    xn_in = nc.declare_dram_parameter("xn", [P, HPC * B * 2 * P], FP8, isOutput=False)
    # wgtc: [Wq,WkT,Wv] x 2 heads | zero-padded cnt blocks for the K-stacked q
    wgt_cols = 6 * P + 2 * HPC * B
    wgt_in = nc.declare_dram_parameter("wgt", [P, wgt_cols], BF16, isOutput=False)
    # cns: bq2 [0:16] | exp bias per s-tile [16:18] | cnt+bv residual [18:34]
    cns_in = nc.declare_dram_parameter("cns", [P, 2 * HPC * B + 2], F32, isOutput=False)
    out_t = nc.declare_dram_parameter("out", [HPC, P, B], F32, isOutput=True)

    JB = HPC * B

    with TileContext(nc) as tc:
        with (
            tc.tile_pool(name="xts", bufs=2) as xtpool,
            tc.tile_pool(name="xns", bufs=2) as xnpool,
            tc.tile_pool(name="small", bufs=2) as spool,
            tc.tile_pool(name="att", bufs=4) as apool,
            tc.tile_pool(name="ps_sc", bufs=2, space="PSUM") as pssc,
            tc.tile_pool(name="ps_sm", bufs=1, space="PSUM") as pssm,
            tc.tile_pool(name="ps_o", bufs=2, space="PSUM") as pso,
            tc.tile_pool(name="ps_w", bufs=1, space="PSUM") as psw,
        ):
            # ---- input DMAs, all issued at t=0 on parallel queues ----
            wgt = spool.tile([P, wgt_cols], BF16, tag="wgt")
            nc.scalar.dma_start(out=wgt[:, :], in_=wgt_in[:, :])
            cns = spool.tile([P, 2 * JB + 2], F32, tag="cns")
            nc.scalar.dma_start(out=cns[:, :], in_=cns_in[:, :])

            xts = []
            for j, eng in zip(range(HPC), (nc.sync, nc.scalar)):
                xt_j = xtpool.tile([P, B * W], FP8, tag=f"xt{j}")
                eng.dma_start(out=xt_j[:, :], in_=xt_in[j, :, :])
                xts.append(xt_j)
            xns = []
            for j in range(HPC):
                xn_j = xnpool.tile([P, B * 2 * P], FP8, tag=f"xn{j}")
                nc.gpsimd.dma_start(
                    out=xn_j[:, :], in_=xn_in[:, j * B * 2 * P:(j + 1) * B * 2 * P]
                )
                xns.append(xn_j)

            # ---- PE warm-up on an engine-local constant ----
            ones = spool.tile([P, P], BF16, tag="ones")
            nc.vector.memset(ones[:, :], 1.0)
            junk_ps = psw.tile([P, P], F32, tag="junk")
            for _ in range(N_WARM):
                nc.tensor.matmul(junk_ps[:, :], ones, ones, start=True, stop=True)

            # ---- q/kq for both heads: K-stacked accumulation ----
            wq = [wgt[:, (3 * j) * P:(3 * j + 1) * P] for j in range(HPC)]
            wkt = [wgt[:, (3 * j + 1) * P:(3 * j + 2) * P] for j in range(HPC)]
            wv = [wgt[:, (3 * j + 2) * P:(3 * j + 3) * P] for j in range(HPC)]
            cnx = [wgt[:, 6 * P + j * JB:6 * P + (j + 1) * JB] for j in range(HPC)]

            qk_ps = pssm.tile([P, 2 * JB], F32, tag="qk")
            nc.tensor.matmul(qk_ps[:, 0:JB], wq[0], cnx[0], start=True, stop=False)
            nc.tensor.matmul(qk_ps[:, 0:JB], wq[1], cnx[1], start=False, stop=True)
            q_sb = spool.tile([P, JB], BF16, tag="q")
            nc.vector.tensor_add(q_sb[:, :], qk_ps[:, 0:JB], cns[:, 0:JB])
            for j in range(HPC):
                nc.tensor.matmul(qk_ps[:, JB + j * B:JB + (j + 1) * B], wkt[j],
                                 q_sb[:, j * B:(j + 1) * B], start=True, stop=True)
            kq_sb = spool.tile([P, JB], BF16, tag="kq")
            nc.vector.tensor_copy(kq_sb[:, :], qk_ps[:, JB:2 * JB])

            for j in range(HPC):
                # scores: per (st, b): [s128, 8] block; useful col of block b is 9b
                a_sb = []
                for st in range(2):
                    sc_ps = pssc.tile([P, B * B], F32, tag="sc")
                    for b in range(B):
                        nc.tensor.matmul(
                            sc_ps[:, b * B:(b + 1) * B],
                            xts[j][:, b * W + st * P: b * W + st * P + P],
                            kq_sb[:, j * B:(j + 1) * B], start=True, stop=True)
                    a = apool.tile([P, B], BF16, tag=f"a{j}{st}")
                    nc.scalar.activation(a[:, :], sc_ps[:, 0:B * B:B + 1], EXP,
                                         bias=cns[:, JB + st:JB + st + 1])
                    a_sb.append(a)

                # denominator broadcast over partitions (accumulate both s-tiles)
                dn_ps = pssm.tile([P, B], F32, tag="dn")
                nc.tensor.matmul(dn_ps[:, :], ones, a_sb[0][:, :],
                                 start=True, stop=False)
                nc.tensor.matmul(dn_ps[:, :], ones, a_sb[1][:, :],
                                 start=False, stop=True)
                rec = spool.tile([P, B], F32, tag="rec")
                nc.vector.reciprocal(rec[:, :], dn_ps[:, :])

                # xa[d,b] = sum_s X[s,d] a[s,b]; two waves so the st0 wave
                # overlaps the st1 exp (per-column accumulation groups)
                xa_ps = pso.tile([P, 2 * B], F32, tag="xa")
                for st in range(2):
                    for b in range(B):
                        c0 = b * 2 * P + st * P
                        nc.tensor.matmul(xa_ps[:, b:b + 1], xns[j][:, c0:c0 + P],
                                         a_sb[st][:, b:b + 1], start=(st == 0),
                                         stop=(st == 1), skip_group_check=True)
                # normalize while copying out of PSUM (xa/den), cast bf16
                xa_sb = spool.tile([P, B], BF16, tag="xa_sb")
                nc.vector.tensor_mul(xa_sb[:, :], xa_ps[:, 0:B], rec[:, :])

                # out[e,b] = Wv[d,e] xa_n[d,b] + (cnt + bv)
                nc.tensor.matmul(xa_ps[:, B:2 * B], wv[j], xa_sb[:, :],
                                 start=True, stop=True)
                fin = spool.tile([P, B], F32, tag=f"fin{j}")
                nc.vector.tensor_add(fin[:, :], xa_ps[:, B:2 * B],
                                     cns[:, JB + 2 + j * B:JB + 2 + (j + 1) * B])
                nc.sync.dma_start(out=out_t[j, :, :], in_=fin[:, :])
    nc.finalize()
    return nc


def _get_nc():
    if "nc" not in _NC_CACHE:
        _NC_CACHE["nc"] = _build_nc()
    return _NC_CACHE["nc"]


def _pos_window_f32():
    """t5_position_bucket(S) with the reference's ops in numpy, sliced to window."""
    if "pos" not in _NC_CACHE:
        NUM_BUCKETS, MAX_DISTANCE = 32, 128
        n = (S - 1) - np.arange(S)
        max_exact = NUM_BUCKETS // 2
        is_small = n < max_exact
        large = max_exact + (
            np.log(np.maximum(n, 1).astype(np.float32) / max_exact)
            / np.log(MAX_DISTANCE / max_exact)
            * (NUM_BUCKETS - max_exact)
        ).astype(np.int32)
        large = np.minimum(large, NUM_BUCKETS - 1)
        pos = np.where(is_small, n, large).astype(np.float32)
        _NC_CACHE["pos"] = pos[CUTOFF:]  # [W]
    return _NC_CACHE["pos"]


def kernel(**inputs) -> np.ndarray:
    t = int(np.asarray(inputs["t"]))
    assert t == T, f"kernel hardcoded for t={T}, got {t}"
    content_t = np.asarray(inputs["content_t"], dtype=np.float32)
    cache = np.asarray(inputs["cache"], dtype=np.float32)
    Wq = np.asarray(inputs["Wq"], dtype=np.float32)
    bq = np.asarray(inputs["bq"], dtype=np.float32)
    Wk = np.asarray(inputs["Wk"], dtype=np.float32)
    Wv = np.asarray(inputs["Wv"], dtype=np.float32)
    bv = np.asarray(inputs["bv"], dtype=np.float32)
    pos_param = np.float32(np.asarray(inputs["pos_param"]))
    # time_mask: the reference's masked_fill chain biases every position equally
    # (softmax-invariant); bk shifts all of a batch's scores equally. Both no-ops.

    posb = (-pos_param * _pos_window_f32()).astype(np.float32)      # [W]

    # window rows per (b, s, h, d), s=0..254 from cache, s=255 = content row
    win = np.empty((B, W, H, P), np.float32)
    win[:, :W - 1] = cache[:, CUTOFF:T, :].reshape(B, W - 1, H, P)
    win[:, W - 1] = content_t.reshape(B, H, P)
    win8 = win.astype(NP_FP8)

    wkt_full = (Wk.transpose(0, 2, 1) / np.float32(np.sqrt(128.0))).astype(np.float32)
    cnt_h = content_t.reshape(B, H, P)
    JB = HPC * B
    wgt_cols = 6 * P + 2 * JB

    in_maps = []
    for c in range(NCORES):
        h0 = HPC * c
        wc = win8[:, :, h0:h0 + HPC, :]                              # [B, W, 2, P]
        # xt[j, d, b*W+s] = wc[b, s, j, d]
        xt_host = np.ascontiguousarray(
            wc.transpose(2, 3, 0, 1).reshape(HPC, P, B * W))
        # xn[s128, ((j*B+b)*2+st)*P+d] = wc[b, st*128+s128, j, d]
        xn_host = np.ascontiguousarray(
            wc.reshape(B, 2, P, HPC, P).transpose(2, 3, 0, 1, 4)
            .reshape(P, HPC * B * 2 * P))
        wgt_host = np.zeros((P, wgt_cols), np.float32)
        for j in range(HPC):
            wgt_host[:, (3 * j) * P:(3 * j + 1) * P] = Wq[h0 + j]
            wgt_host[:, (3 * j + 1) * P:(3 * j + 2) * P] = wkt_full[h0 + j]
            wgt_host[:, (3 * j + 2) * P:(3 * j + 3) * P] = Wv[h0 + j]
            # zero-padded cnt block for the K-stacked q: block j holds cnt_j in
            # its own (j,b) columns, zeros elsewhere
            wgt_host[:, 6 * P + j * JB + j * B:6 * P + j * JB + (j + 1) * B] = \
                cnt_h[:, h0 + j, :].T
        cns_host = np.empty((P, 2 * JB + 2), np.float32)
        for j in range(HPC):
            cns_host[:, j * B:(j + 1) * B] = bq[h0 + j][:, None]
            cns_host[:, JB + 2 + j * B:JB + 2 + (j + 1) * B] = (
                cnt_h[:, h0 + j, :] + bv[h0 + j][None, :]).T
        cns_host[:, JB] = posb[0:P]
        cns_host[:, JB + 1] = posb[P:W]
        in_maps.append({
            "xt": xt_host,
            "xn": xn_host,
            "wgt": wgt_host.astype(NP_BF16),
            "cns": cns_host,
        })

    nc = _get_nc()
    res = run_bass_kernel_spmd(nc, in_maps, list(range(NCORES)), **_RUN_KWARGS)
    _NC_CACHE["last_results"] = res
    # out[j, e, b] per core -> out_full[b, (2c+j)*128+e]
    out_full = np.empty((B, H * P), np.float32)
    for c in range(NCORES):
        oc = np.asarray(res.results[c]["out"])
        for j in range(HPC):
            out_full[:, (HPC * c + j) * P:(HPC * c + j + 1) * P] = oc[j].T
    return out_full


_RUN_KWARGS = {}  # test harness may set {"trace": True, "tmpdir": ...}
